# revision 6
# baseline (speedup 1.0000x reference)
"""HGCN (hypergraph conv net) Trainium2 kernel over 8 NeuronCores.

Math: out = log_softmax(pool_mean(h2) @ Wfc + bfc) where
  h_k = relu((D^-1 H B^-1 H^T h_{k-1}) @ Wk + bk)

Design (nodes sharded 12500/core; all index-derived data precomputed on CPU):
  stage A (node->edge): incidence entries grouped by 128-edge window; per
    window, bins of <=128 entries become one-hot S tiles (DVE iota-compare)
    and psum[slot, ch] += S^T gather(x rows) on PE -- the window tile lands
    dense, so there are NO SWDGE scatters, no dense memsets, and per-2560-row
    slices store contiguously to DRAM overlapped with the rest of stage A.
  exchange: ReduceScatter(add, bf16) -> fp8 convert of the reduced slice ->
    AllGather in fp8 (half the wire bytes of the bf16 AllReduce).
  stage B (edge->node): gathers 256B fp8 edge-PAIR rows from e_full8; a
    widened S [128, 256] whose column encodes parity*128+slot selects the
    right half via two accumulating matmuls; psum is [ch, slot] so it feeds
    the fused W-phase matmul directly (no PE transposes, no a_dense round
    trip). Layer 1 stores h rows for the layer-2 gather; layer 2 fuses the
    mean-pool psum. Tail: pooled AllGather + local add, fc, log_softmax.

SPMD: one program for all cores; bins-per-window is the max over cores
(a degree-balanced shared edge permutation plus per-core node permutations
keep that padding small).
"""
import numpy as np

import concourse.bacc as bacc
import concourse.tile as tile
import concourse.mybir as mybir
from concourse.bass_utils import run_bass_kernel_spmd

NCORES = 8
N_NODES = 100000
N_EDGES = 20000
NNZ = 600000
N_GRAPHS = 64
C = 128
OUT_C = 16
NPC = N_NODES // NCORES          # 12500
NPC_PAD = 12544                  # 98 * 128
NW_N = NPC_PAD // 128            # 98 node windows
E_PAD = 20480                    # 160 * 128, divisible by 8*128
NW_E = E_PAD // 128              # 160 edge windows
ESL = E_PAD // NCORES            # 2560 rows per RS slice
WSL = NW_E // NCORES             # 20 windows per slice
GCT = 8                          # gather tiles (bins) per dma_gather call
E_PAIRS = E_PAD // 2             # fp8 e rows are edge pairs (256B)
NQ = 4

F32 = mybir.dt.float32
BF16 = mybir.dt.bfloat16
I16 = mybir.dt.int16
F8 = mybir.dt.float8e4


# ----------------------------------------------------------------- CPU pack
def _balance(ids, weights, n_buckets, cap):
    """LPT: deal ids (sorted by weight desc) into n_buckets of <=cap ids.
    Returns pos[id] = bucket*cap + slot."""
    order = np.argsort(-weights, kind="stable")
    loads = np.zeros(n_buckets)
    fill = np.zeros(n_buckets, np.int64)
    pos = np.empty(len(ids), np.int64)
    # heap-free LPT: iterate in chunks using argmin
    import heapq
    heap = [(0.0, 0, b) for b in range(n_buckets)]
    heapq.heapify(heap)
    for i in order:
        while True:
            load, f, b = heapq.heappop(heap)
            if fill[b] < cap:
                break
        pos[i] = b * cap + fill[b]
        fill[b] += 1
        loads[b] = load + weights[i]
        heapq.heappush(heap, (loads[b], int(fill[b]), b))
    return pos


def _bins_per_window(dstw_counts_by_core, n_windows):
    """B[w] = max over cores of ceil(count/128), min 1."""
    B = np.ones(n_windows, np.int64)
    for counts in dstw_counts_by_core:
        B = np.maximum(B, -(-counts // 128))
    return B


def _pack_windows(dst_w, slot, gat, wgt, n_windows, B):
    """Pack entries into the uniform (window, bin) layout.

    dst_w: window of each entry; slot: 0..127 within window; gat: gather row;
    wgt: weight.  B[w]: bins allotted to window w (uniform across cores).
    Returns gidx[NT,128] int32, colid[NT,128] f32, w[NT,128] f32 with
    NT = sum(B); bin t covers window w for t in [off[w], off[w]+B[w]).
    """
    NT = int(B.sum())
    off = np.zeros(n_windows + 1, np.int64)
    np.cumsum(B, out=off[1:])
    gidx = np.zeros((NT, 128), np.int32)
    colid = np.full((NT, 128), -1.0, np.float32)
    wv = np.zeros((NT, 128), np.float32)
    order = np.argsort(dst_w, kind="stable")
    dw = dst_w[order]; sl = slot[order]; ga = gat[order]; wg = wgt[order]
    starts = np.searchsorted(dw, np.arange(n_windows))
    ends = np.searchsorted(dw, np.arange(n_windows) + 1)
    for w in range(n_windows):
        s, e = int(starts[w]), int(ends[w])
        n = e - s
        assert n <= B[w] * 128, (w, n, B[w])
        t0 = int(off[w])
        for j in range(-(-n // 128) if n else 0):
            a = s + j * 128
            b = min(s + (j + 1) * 128, e)
            m = b - a
            gidx[t0 + j, :m] = ga[a:b]
            colid[t0 + j, :m] = sl[a:b]
            wv[t0 + j, :m] = wg[a:b]
    return gidx, colid, wv, NT


def _wrap16(tokens):
    a = tokens.reshape(-1, 16).T.astype(np.int16)
    return np.tile(a, (8, 1)).copy()


def _prep_inputs(x, node_idx, edge_idx, batch, W1, b1, W2, b2, Wfc, bfc):
    node_idx = np.asarray(node_idx).astype(np.int64)
    edge_idx = np.asarray(edge_idx).astype(np.int64)
    batch = np.asarray(batch).astype(np.int64)
    x = np.asarray(x, np.float32)

    De = np.bincount(edge_idx, minlength=N_EDGES).astype(np.float32)
    Dn = np.bincount(node_idx, minlength=N_NODES).astype(np.float32)
    Binv = np.where(De > 0, 1.0 / np.maximum(De, 1), 0.0).astype(np.float32)
    Dinv = np.where(Dn > 0, 1.0 / np.maximum(Dn, 1), 0.0).astype(np.float32)
    cnt = np.bincount(batch, minlength=N_GRAPHS).astype(np.float32)
    cntinv = (1.0 / np.maximum(cnt, 1.0)).astype(np.float32)

    # shared edge permutation: epos[e] = dense row in e tensors
    epos = _balance(np.arange(N_EDGES), De, NW_E, 128)

    owner = node_idx // NPC
    # per-core node permutation (local), degree balanced over 98 windows
    npos_l = []
    per_core = []
    for c in range(NCORES):
        m = owner == c
        ln = node_idx[m] - c * NPC
        le = edge_idx[m]
        deg = np.bincount(ln, minlength=NPC).astype(np.float32)
        npos = _balance(np.arange(NPC), deg, NW_N, 128)
        npos_l.append(npos)
        per_core.append((ln, le))

    # uniform bins per window
    cntsA, cntsB = [], []
    for c in range(NCORES):
        ln, le = per_core[c]
        ew = epos[le] // 128
        nw = npos_l[c][ln] // 128
        cntsA.append(np.bincount(ew, minlength=NW_E))
        cntsB.append(np.bincount(nw, minlength=NW_N))
    B_A = _bins_per_window(cntsA, NW_E)
    B_B = _bins_per_window(cntsB, NW_N)
    # pad NT to a multiple of GCT by growing the last window
    B_A[-1] += -(-int(B_A.sum()) // GCT) * GCT - int(B_A.sum())
    B_B[-1] += -(-int(B_B.sum()) // GCT) * GCT - int(B_B.sum())
    NT_A, NT_B = int(B_A.sum()), int(B_B.sum())

    in_maps = []
    meta = dict(NT_A=NT_A, NT_B=NT_B, B_A=tuple(int(v) for v in B_A),
                B_B=tuple(int(v) for v in B_B))
    for c in range(NCORES):
        ln, le = per_core[c]
        npos = npos_l[c]
        ep = epos[le]
        npp = npos[ln]
        gA, cA, wA, _ = _pack_windows(ep // 128, ep % 128, npp, Binv[le],
                                      NW_E, B_A)
        gB, cB, wB, _ = _pack_windows(npp // 128,
                                      (ep % 2) * 128 + (npp % 128), ep // 2,
                                      Dinv[ln + c * NPC], NW_N, B_B)
        xc = np.zeros((NPC_PAD, C), np.float32)
        xc[npos] = x[c * NPC:(c + 1) * NPC]
        batchcol = np.full((NW_N, 128), -1.0, np.float32)
        batchcol.reshape(-1)[npos] = batch[c * NPC:(c + 1) * NPC]
        im = {
            "x": xc.astype(mybir.dt.np(BF16)),
            "gA": _wrap16(gA.reshape(-1)), "gB": _wrap16(gB.reshape(-1)),
            "colA": cA.T.copy(), "wA": wA.T.copy(),
            "colB": cB.T.copy(), "wB": wB.T.copy(),
            "batchcol": batchcol.T.copy(),
            "W1": np.asarray(W1, np.float32).astype(mybir.dt.np(BF16)),
            "W2": np.asarray(W2, np.float32).astype(mybir.dt.np(BF16)),
            "Wfc": np.asarray(Wfc, np.float32),
            "b1rep": np.tile(np.asarray(b1, np.float32)[None, :], (128, 1)),
            "b2rep": np.tile(np.asarray(b2, np.float32)[None, :], (128, 1)),
            "bfcrep": np.tile(np.asarray(bfc, np.float32)[None, :],
                              (N_GRAPHS, 1)),
            "cntinvrep": np.tile(cntinv[None, :], (128, 1)),
        }
        in_maps.append(im)
    return in_maps, meta


# ----------------------------------------------------------------- device
def _build(meta):
    NT_A, NT_B = meta["NT_A"], meta["NT_B"]
    B_A, B_B = meta["B_A"], meta["B_B"]

    nc = bacc.Bacc("TRN2", target_bir_lowering=False, debug=False,
                   num_devices=NCORES, num_swdge_queues=NQ)

    def din(name, shape, dt):
        return nc.dram_tensor(name, shape, dt, kind="ExternalInput")

    x_t = din("x", [NPC_PAD, C], BF16)
    gA_t = din("gA", [128, NT_A * 8], I16)
    gB_t = din("gB", [128, NT_B * 8], I16)
    colA_t = din("colA", [128, NT_A], F32)
    wA_t = din("wA", [128, NT_A], F32)
    colB_t = din("colB", [128, NT_B], F32)
    wB_t = din("wB", [128, NT_B], F32)
    batchcol_t = din("batchcol", [128, NW_N], F32)
    W1_t = din("W1", [C, C], BF16)
    W2_t = din("W2", [C, C], BF16)
    Wfc_t = din("Wfc", [C, OUT_C], F32)
    b1rep_t = din("b1rep", [128, C], F32)
    b2rep_t = din("b2rep", [128, C], F32)
    bfcrep_t = din("bfcrep", [N_GRAPHS, OUT_C], F32)
    cntinvrep_t = din("cntinvrep", [128, N_GRAPHS], F32)

    out_t = nc.dram_tensor("out", [N_GRAPHS, OUT_C], F32, kind="ExternalOutput")

    h1 = nc.dram_tensor("h1", [NPC_PAD, C], BF16)
    e_part = [nc.dram_tensor(f"e_part{i}", [E_PAD, C], BF16) for i in range(2)]
    e_red = [nc.dram_tensor(f"e_red{i}", [ESL, C], BF16) for i in range(2)]
    e_red8 = [nc.dram_tensor(f"e_red8{i}", [ESL, C], F8) for i in range(2)]
    e_full8 = [nc.dram_tensor(f"e_full8{i}", [E_PAIRS, 2 * C], F8,
                              addr_space="Shared") for i in range(2)]
    pool_cat = nc.dram_tensor("pool_cat", [NCORES * 128, N_GRAPHS], F32,
                              addr_space="Shared")
    pool_part = nc.dram_tensor("pool_part", [128, N_GRAPHS], F32)

    with tile.TileContext(nc) as tc:
        with (
            tc.tile_pool(name="res", bufs=1) as res,
            tc.tile_pool(name="gp", bufs=3) as gp,
            tc.tile_pool(name="sp", bufs=6) as sp,
            tc.tile_pool(name="wp", bufs=3) as wp,
            tc.tile_pool(name="ap", bufs=3) as apool,
            tc.tile_pool(name="ps", bufs=2, space="PSUM") as ps,
            tc.tile_pool(name="psw", bufs=1, space="PSUM") as psw,
            tc.tile_pool(name="pspool", bufs=1, space="PSUM") as pspool,
        ):
            def rload(t, shape, dt):
                tl = res.tile(shape, dt, tag=t.name)
                nc.sync.dma_start(tl[:], t.ap())
                return tl

            gA = rload(gA_t, [128, NT_A * 8], I16)
            gB = rload(gB_t, [128, NT_B * 8], I16)
            colA = rload(colA_t, [128, NT_A], F32)
            wA = rload(wA_t, [128, NT_A], F32)
            colB = rload(colB_t, [128, NT_B], F32)
            wB = rload(wB_t, [128, NT_B], F32)
            batchcol = rload(batchcol_t, [128, NW_N], F32)
            W1 = rload(W1_t, [C, C], BF16)
            W2 = rload(W2_t, [C, C], BF16)
            Wfc = rload(Wfc_t, [C, OUT_C], F32)
            b1rep = rload(b1rep_t, [128, C], F32)
            b2rep = rload(b2rep_t, [128, C], F32)
            bfcrep = rload(bfcrep_t, [N_GRAPHS, OUT_C], F32)
            cntinvrep = rload(cntinvrep_t, [128, N_GRAPHS], F32)

            iota = res.tile([128, 128], BF16, tag="iota")
            nc.gpsimd.iota(iota[:], [[1, 128]], channel_multiplier=0,
                           allow_small_or_imprecise_dtypes=True)
            iota2 = res.tile([128, 256], BF16, tag="iota2")
            nc.gpsimd.iota(iota2[:], [[1, 256]], channel_multiplier=0,
                           allow_small_or_imprecise_dtypes=True)
            # resident e window buffer [slot, window, ch]
            e_sb = res.tile([128, NW_E, 128], BF16, tag="e_sb")

            qn = [0]

            def next_q():
                q = qn[0] % NQ
                qn[0] += 1
                return q

            def stage_a(src_ap, e_dst, layer):
                gts = {}
                for k in range(NT_A // GCT):
                    gt = gp.tile([128, GCT, 128], BF16, tag="gA")
                    nc.gpsimd.dma_gather(
                        gt[:], src_ap, gA[:, k * GCT * 8:(k + 1) * GCT * 8],
                        GCT * 128, GCT * 128, C, queue_num=next_q())
                    gts[k] = gt
                    _drain_a(gts, e_dst, done=False)
                _drain_a(gts, e_dst, done=True)

            # windows are packed t-contiguously; walk bins against gather call
            # availability: window w occupies bins [offA[w], offA[w]+B_A[w]).
            offA = np.zeros(NW_E + 1, np.int64)
            np.cumsum(np.asarray(B_A), out=offA[1:])
            offB = np.zeros(NW_N + 1, np.int64)
            np.cumsum(np.asarray(B_B), out=offB[1:])

            state_a = {"w": 0}

            def _drain_a(gts, e_dst, done):
                avail = (max(gts.keys()) + 1) * GCT if gts else 0
                w = state_a["w"]
                while w < NW_E and (offA[w + 1] <= avail):
                    acc = ps.tile([128, 128], F32, tag="accA")
                    nb = int(offA[w + 1] - offA[w])
                    for j in range(nb):
                        t = int(offA[w]) + j
                        S = sp.tile([128, 128], BF16, tag="S")
                        nc.vector.tensor_scalar(
                            S[:], iota[:], colA[:, t:t + 1], wA[:, t:t + 1],
                            op0=mybir.AluOpType.is_equal,
                            op1=mybir.AluOpType.mult)
                        gt = gts[t // GCT]
                        nc.tensor.matmul(acc[:], S[:], gt[:, t % GCT, :],
                                         start=(j == 0), stop=(j == nb - 1))
                    nc.scalar.copy(e_sb[:, w, :], acc[:])
                    if (w + 1) % WSL == 0:
                        o = w // WSL
                        dst = e_dst.ap()[o * ESL:(o + 1) * ESL, :].rearrange(
                            "(w p) c -> p w c", p=128)
                        nc.sync.dma_start(dst,
                                          e_sb[:, o * WSL:(o + 1) * WSL, :])
                    w += 1
                state_a["w"] = w
                if done:
                    assert w == NW_E
                    state_a["w"] = 0
                    # free consumed gather tiles
                    gts.clear()

            state_b = {"w": 0}

            def stage_b(efull_ap, W, brep, h_out, pool_acc):
                gts = {}
                for k in range(NT_B // GCT):
                    gt = gp.tile([128, GCT, 2 * C], F8, tag="gB")
                    nc.gpsimd.dma_gather(
                        gt[:], efull_ap, gB[:, k * GCT * 8:(k + 1) * GCT * 8],
                        GCT * 128, GCT * 128, 2 * C, queue_num=next_q())
                    gts[k] = gt
                    _drain_b(gts, W, brep, h_out, pool_acc, done=False)
                _drain_b(gts, W, brep, h_out, pool_acc, done=True)

            def _drain_b(gts, W, brep, h_out, pool_acc, done):
                avail = (max(gts.keys()) + 1) * GCT if gts else 0
                w = state_b["w"]
                while w < NW_N and (offB[w + 1] <= avail):
                    acc = ps.tile([128, 128], F32, tag="accB")
                    nb = int(offB[w + 1] - offB[w])
                    for j in range(nb):
                        t = int(offB[w]) + j
                        S = sp.tile([128, 2 * C], BF16, tag="S")
                        nc.vector.tensor_scalar(
                            S[:], iota2[:], colB[:, t:t + 1],
                            wB[:, t:t + 1],
                            op0=mybir.AluOpType.is_equal,
                            op1=mybir.AluOpType.mult)
                        gt = gts[t // GCT]
                        nc.tensor.matmul(acc[:], gt[:, t % GCT, 0:C],
                                         S[:, 0:C],
                                         start=(j == 0), stop=False)
                        nc.tensor.matmul(acc[:], gt[:, t % GCT, C:2 * C],
                                         S[:, C:2 * C],
                                         start=False, stop=(j == nb - 1))
                    ats = apool.tile([128, 128], BF16, tag="ats")
                    nc.scalar.copy(ats[:], acc[:])
                    hp = psw.tile([128, 128], F32, tag="hp")
                    nc.tensor.matmul(hp[:], ats[:], W[:], start=True,
                                     stop=True)
                    ht = wp.tile([128, 128], BF16, tag="ht")
                    nc.vector.tensor_tensor(ht[:], hp[:], brep[:],
                                            op=mybir.AluOpType.add)
                    nc.vector.tensor_scalar_max(ht[:], ht[:], 0.0)
                    if h_out is not None:
                        nc.sync.dma_start(h_out.ap()[w * 128:(w + 1) * 128, :],
                                          ht[:])
                    if pool_acc is not None:
                        Sp = sp.tile([128, N_GRAPHS], BF16, tag="Spool")
                        nc.vector.tensor_scalar(
                            Sp[:], iota[:, 0:N_GRAPHS], batchcol[:, w:w + 1],
                            None, op0=mybir.AluOpType.is_equal)
                        nc.tensor.matmul(pool_acc, ht[:], Sp[:],
                                         start=(w == 0), stop=(w == NW_N - 1),
                                         skip_group_check=True)
                    w += 1
                state_b["w"] = w
                if done:
                    assert w == NW_N
                    state_b["w"] = 0
                    gts.clear()

            def exchange(i):
                nc.gpsimd.collective_compute(
                    "ReduceScatter", mybir.AluOpType.add,
                    replica_groups=[list(range(NCORES))],
                    ins=[e_part[i].ap()], outs=[e_red[i].ap()])
                # bf16 -> fp8 for a half-size AllGather
                rt = wp.tile([128, WSL, C], BF16, tag="rt")
                nc.sync.dma_start(
                    rt[:], e_red[i].ap().rearrange("(w p) c -> p w c", p=128))
                rt8 = wp.tile([128, WSL, C], F8, tag="rt8")
                nc.vector.tensor_copy(rt8[:], rt[:])
                nc.sync.dma_start(
                    e_red8[i].ap().rearrange("(w p) c -> p w c", p=128),
                    rt8[:])
                nc.gpsimd.collective_compute(
                    "AllGather", mybir.AluOpType.bypass,
                    replica_groups=[list(range(NCORES))],
                    ins=[e_red8[i].ap()], outs=[e_full8[i].ap()])

            # ---------------- layer 1
            stage_a(x_t.ap(), e_part[0], 0)
            exchange(0)
            stage_b(e_full8[0].ap(), W1, b1rep, h1, None)

            # ---------------- layer 2
            stage_a(h1.ap(), e_part[1], 1)
            exchange(1)
            pool_acc = pspool.tile([128, N_GRAPHS], F32, tag="pool")
            stage_b(e_full8[1].ap(), W2, b2rep, None, pool_acc[:])

            # ---------------- pooling + fc + log_softmax
            pt = wp.tile([128, N_GRAPHS], F32, tag="pt")
            nc.vector.tensor_tensor(pt[:], pool_acc[:], cntinvrep[:],
                                    op=mybir.AluOpType.mult)
            nc.sync.dma_start(pool_part.ap(), pt[:])
            nc.gpsimd.collective_compute(
                "AllGather", mybir.AluOpType.bypass,
                replica_groups=[list(range(NCORES))],
                ins=[pool_part.ap()], outs=[pool_cat.ap()])
            pcat = wp.tile([128, NCORES, N_GRAPHS], F32, tag="pcat")
            nc.sync.dma_start(
                pcat[:], pool_cat.ap().rearrange("(k p) g -> p k g", p=128))
            ptf = wp.tile([128, N_GRAPHS], F32, tag="ptf")
            nc.vector.tensor_tensor(ptf[:], pcat[:, 0, :], pcat[:, 1, :],
                                    op=mybir.AluOpType.add)
            for k in range(2, NCORES):
                nc.vector.tensor_tensor(ptf[:], ptf[:], pcat[:, k, :],
                                        op=mybir.AluOpType.add)
            lg = pspool.tile([N_GRAPHS, OUT_C], F32, tag="lg")
            nc.tensor.matmul(lg[:], ptf[:], Wfc[:], start=True, stop=True)
            z = wp.tile([N_GRAPHS, OUT_C], F32, tag="z")
            nc.vector.tensor_tensor(z[:], lg[:], bfcrep[:],
                                    op=mybir.AluOpType.add)
            mx = wp.tile([N_GRAPHS, 1], F32, tag="mx")
            nc.vector.tensor_reduce(mx[:], z[:], mybir.AxisListType.X,
                                    mybir.AluOpType.max)
            zs = wp.tile([N_GRAPHS, OUT_C], F32, tag="zs")
            nc.vector.tensor_scalar(zs[:], z[:], mx[:], None,
                                    op0=mybir.AluOpType.subtract)
            ez = wp.tile([N_GRAPHS, OUT_C], F32, tag="ez")
            se = wp.tile([N_GRAPHS, 1], F32, tag="se")
            nc.scalar.activation(ez[:], zs[:], mybir.ActivationFunctionType.Exp,
                                 accum_out=se[:])
            lse = wp.tile([N_GRAPHS, 1], F32, tag="lse")
            nc.scalar.activation(lse[:], se[:], mybir.ActivationFunctionType.Ln)
            outz = wp.tile([N_GRAPHS, OUT_C], F32, tag="outz")
            nc.vector.tensor_scalar(outz[:], zs[:], lse[:], None,
                                    op0=mybir.AluOpType.subtract)
            nc.sync.dma_start(out_t.ap(), outz[:])

    nc.compile()
    _fix_swdge_queues(nc)
    return nc


def _fix_swdge_queues(nc):
    """queue_num := scheduled DMASW lane % NQ so each DMASW semaphore is
    only ever updated from one SWDGE queue."""
    import bass_rust
    lo = bass_rust.dmasw_start_idx
    hi = lo + bass_rust.NUM_SWDGE_GLOBAL_SEMS
    for fn in nc.m.functions:
        for blk in fn.blocks:
            for inst in blk.instructions:
                if getattr(inst, "queue_num", None) is None:
                    continue
                proc = getattr(inst, "bass_scheduled_proc", None)
                if proc is not None and lo <= proc < hi:
                    inst.queue_num = (proc - lo) % NQ


_CACHE = {}


def kernel(**inputs) -> np.ndarray:
    in_maps, meta = _prep_inputs(**inputs)
    key = (meta["NT_A"], meta["NT_B"])
    if key not in _CACHE:
        _CACHE[key] = _build(meta)
    nc = _CACHE[key]
    res = run_bass_kernel_spmd(nc, in_maps, core_ids=list(range(NCORES)))
    return res.results[0]["out"].astype(np.float32)
